# revision 25
# baseline (speedup 1.0000x reference)
"""Self-contained Trainium2 Bass kernel for a single attention head.

Problem: B=8, S=2048, E=1024, D=64 (fp32 in/out).
  q = query @ Wq.T + bq ; k, v likewise
  out = softmax(mask(q @ k.T / sqrt(D))) @ v
  mask = query_mask[:, :, None] * key_mask[:, None, :]; query_mask is all-ones
  per the problem spec (fill="ones").

Sharding: pure data-parallel, one batch element per NeuronCore (8 cores).

Key ideas:
  - fp16 compute with fp32 PSUM accumulation.  (fp8 was measured and fails:
    quantization noise on q/k/v/p does NOT average out through the softmax
    weighted sum -- each gives ~4-7% rel error vs the 2e-2 budget.)
  - Host compacts away masked key columns; S_k shrinks 2048 -> ~1100, padded
    to a multiple of 128.  Pad columns get mask bias -30000 -> exp == 0.
  - Score matmuls contract over only D=64, so two key tiles are packed into
    PE row-groups (0,0)/(64,0) and run concurrently: qT/kT carry duplicate
    data in partitions 64:128, written for free by col-group-packed
    projection matmuls ((0,0)+(0,64) share one moving stream).
  - Softmax: key dim on partitions; key mask is a per-partition bias on the
    ACT exp; the denominator falls out of AV as a 65th row (X = [v | ones]).
    No row-max subtraction (scores stay within +-~6).
  - The kernel ships the UNNORMALIZED [65, S] numerator+denominator to DRAM
    as one fat contiguous fp16 blob; the host does the divide + transpose.
    This removes all finalize transposes/reciprocals and the slow strided
    output DMA from the hot loop.
  - Staging blobs are laid out host-side exactly as SBUF wants them
    ([partition, e-block * cols]) so every stage DMA is a contiguous 1:1
    copy, issued in arrival order on the GpSimd SWDGE ring:
    q-half0 in two, k in two, q-half1 in two, v in two.
  - Emission is hand-pipelined around the in-order engines: the first score
    pair starts on a 512-wide chunk as soon as q cols 0:512 + k tiles 0:4
    land; projection/x-transpose/AV-half0 items are pumped into the
    ACT-paced score-pair gaps; AV half1 and the output DMAs ride the tail.
"""

from contextlib import ExitStack

import numpy as np
import ml_dtypes

import concourse.bass as bass
import concourse.mybir as mybir
import concourse.tile as tile
from concourse import bacc
from concourse.bass_utils import run_bass_kernel_spmd
from concourse.masks import make_identity

FP16 = mybir.dt.float16
FP8 = mybir.dt.float8e4
F32 = mybir.dt.float32
E4M3 = ml_dtypes.float8_e4m3fn

N_CORES = 8
B, S, E, D = 8, 2048, 1024, 64
P = 128
NE = E // P            # 8 contraction tiles
NH = 2                 # halves (PSUM capacity)
HI = S // NH           # 1024 query positions per half
NC = 512               # matmul free-dim chunk (one PSUM bank of f32)
SCALE = 1.0 / np.sqrt(np.float32(D))
MASK_NEG = -30000.0


def _chunks(total, step):
    out = []
    o = 0
    while o < total:
        out.append((o, min(step, total - o)))
        o += step
    return out


def _build(tc: tile.TileContext, ins: dict, out_d: bass.AP, ctx, sk2: int):
    nc = tc.nc
    nj = sk2 // P

    consts = ctx.enter_context(tc.tile_pool(name="consts", bufs=1))
    stage = ctx.enter_context(tc.tile_pool(name="stage", bufs=1))
    proj = ctx.enter_context(tc.tile_pool(name="proj", bufs=1))
    xpool = ctx.enter_context(tc.tile_pool(name="xpool", bufs=16))
    ppool = ctx.enter_context(tc.tile_pool(name="ppool", bufs=18))
    finp = ctx.enter_context(tc.tile_pool(name="finp", bufs=2))
    ps_mm = ctx.enter_context(tc.tile_pool(name="ps_mm", bufs=2, space="PSUM"))
    ps_px = ctx.enter_context(tc.tile_pool(name="ps_px", bufs=2, space="PSUM"))
    ps_acc = ctx.enter_context(tc.tile_pool(name="ps_acc", bufs=1, space="PSUM"))

    # --- constants (tiny, issued first on the HWDGE ring) ---------------
    c16 = consts.tile([P, 3 * NE * D], FP16, tag="c16")
    c32 = consts.tile([P, nj + 3], F32, tag="c32")
    nc.sync.dma_start(out=c16[:], in_=ins["c16"][:])
    nc.sync.dma_start(out=c32[:], in_=ins["c32"][:])
    wq = c16[:, 0:NE * D]
    wk = c16[:, NE * D:2 * NE * D]
    wv = c16[:, 2 * NE * D:3 * NE * D]
    mb = c32[:, 0:nj]
    bq = c32[:, nj:nj + 1]          # biases duplicated into rows 64:128
    bk = c32[:, nj + 1:nj + 2]
    bv = c32[0:D, nj + 2:nj + 3]

    # identity (for the v transposes and the HAM-warming pad matmuls)
    # runs on the gpsimd queue; emit it before the staging DMA instructions
    # so it isn't stuck behind them.
    warm16 = consts.tile([P, 16], FP16, tag="warm16")
    nc.vector.memset(warm16[:], 0.0)
    ident16 = consts.tile([P, P], FP16, tag="ident16")
    warm = consts.tile([P, 16], F32, tag="warm")
    make_identity(nc, ident16[:])
    nc.vector.memset(warm[:], 0.0)
    nc.scalar.activation(warm[:], warm[:], mybir.ActivationFunctionType.Exp)

    # The PE's HAM clock gate re-throttles to 1.2 GHz when it sees idle
    # windows; this ACT/DMA-paced kernel has many 1-3us PE gaps.  pepad()
    # emits dependency-free 128-col dummy matmuls (~57ns each, ~93% array
    # duty) that soak up known PE-idle zones and keep the clock at 2.4 GHz.
    wps = ps_px.tile([P, NC], F32, tag="px", name="wps")

    def pepad(n):
        for _ in range(n):
            nc.tensor.matmul(wps[0:16, 0:P], warm16[:], ident16[:, 0:P],
                             start=True, stop=True)

    pepad(60)   # pre-warm burst while the first staging DMAs fly

    # --- staged inputs, in arrival order on the SWDGE ring ---------------
    # The first pieces are small (256-col) so the first projection and score
    # matmuls start as early as possible.
    HC = NC // 2
    KA = min(NC, sk2)
    KB = sk2 - KA
    QA = NC
    QB = HI - QA
    vsplit = _chunks(sk2, NC)          # v chunk per stage piece
    stg = {}
    pieces = ([("qst0a1", HC), ("qst0a2", HC),
               ("ksta1", min(HC, KA)), ("ksta2", max(KA - HC, 0)),
               ("qst0b", QB), ("kstb", KB),
               ("qst1a", QA), ("qst1b", QB)] +
              [(f"vst{i}", n) for i, (o, n) in enumerate(vsplit)])
    for name, n in pieces:
        if n == 0:
            continue
        stg[name] = stage.tile([P, NE * n], FP16, tag=name, name=name)
        nc.gpsimd.dma_start(out=stg[name][:], in_=ins[name][:])

    # persistent projected tensors.  qT/kT rows 64:128 are duplicates of
    # rows 0:64 (written by the col-group-packed projections) so the packed
    # score matmuls can stream/load from the upper partitions.
    qT_sb = proj.tile([P, S], FP16, tag="qT_sb")
    kT_sb = proj.tile([P, sk2], FP16, tag="kT_sb")
    vT_sb = proj.tile([D, sk2], FP16, tag="vT_sb")

    def proj_mms(ps, w, src, e0, e1, estride, soff, n, dup):
        for e in range(e0, e1):
            sl = src[:, e * estride + soff:e * estride + soff + n]
            wt = w[:, e * D:(e + 1) * D]
            nc.tensor.matmul(ps[0:D, 0:n], wt, sl,
                             start=(e == 0), stop=(e == NE - 1),
                             tile_position=(0, 0))
            if dup:
                nc.tensor.matmul(ps[D:P, 0:n], wt, sl,
                                 start=(e == 0), stop=(e == NE - 1),
                                 tile_position=(0, 64))

    def proj_chunk(dst, w, bias_ap, src, estride, soff, doff, n, dup):
        ps = ps_px.tile([P, NC], F32, tag="px",
                        name=f"ps_{dst.tensor.name}_{doff}")
        proj_mms(ps, w, src, 0, NE, estride, soff, n, dup)
        rows = P if dup else D
        nc.vector.tensor_scalar_add(
            dst[0:rows, doff:doff + n], ps[0:rows, 0:n], bias_ap)

    def proj_pumps(chunks):
        """Split each projection chunk into two 4-e-tile pump items sized to
        fit the per-pair PE idle gap of the ACT-paced score loops."""
        items = []
        for (dst, w, bias_ap, src, estride, soff, doff, n, dup) in chunks:
            st = {}

            def sub(ehalf, st=st, dst=dst, w=w, bias_ap=bias_ap, src=src,
                    estride=estride, soff=soff, doff=doff, n=n, dup=dup):
                if ehalf == 0:
                    st["ps"] = ps_px.tile(
                        [P, NC], F32, tag="px",
                        name=f"psp_{dst.tensor.name}_{doff}")
                proj_mms(st["ps"], w, src, ehalf * (NE // 2),
                         (ehalf + 1) * (NE // 2), estride, soff, n, dup)
                if ehalf == 1:
                    rows = P if dup else D
                    nc.vector.tensor_scalar_add(
                        dst[0:rows, doff:doff + n],
                        st["ps"][0:rows, 0:n], bias_ap)

            items.append(lambda s=sub: s(0))
            items.append(lambda s=sub: s(1))
        return items

    # ---- attention helpers ---------------------------------------------
    sst = {}

    def scores_mms(h, j0, c0, c1):
        """Packed score matmuls for the pair (j0, j0+1): key tile j0 on PE
        row-group (0,0), j0+1 on (64,0), running concurrently.  Emits query
        chunks [c0, c1)."""
        js = [j0] + ([j0 + 1] if j0 + 1 < nj else [])
        for idx, j in enumerate(js):
            if (h, j) not in sst:
                sst[(h, j)] = ps_mm.tile([P, HI], F32, tag="ps_mm",
                                         name=f"ssT_{h}_{j}")
        for c in range(c0, c1):
            for idx, j in enumerate(js):
                r = idx * D
                nc.tensor.matmul(
                    sst[(h, j)][:, c * NC:(c + 1) * NC],
                    kT_sb[r:r + D, j * P:(j + 1) * P],
                    qT_sb[r:r + D, h * HI + c * NC:h * HI + (c + 1) * NC],
                    start=True, stop=True,
                    tile_position=(r, 0),
                )

    def exps(h, j0, pms, c0, c1):
        js = [j0] + ([j0 + 1] if j0 + 1 < nj else [])
        for j in js:
            if j not in pms:
                pms[j] = ppool.tile([P, HI], FP16, tag="pm",
                                    name=f"pm_{h}_{j}")
            nc.scalar.activation(pms[j][:, c0 * NC:c1 * NC],
                                 sst[(h, j)][:, c0 * NC:c1 * NC],
                                 mybir.ActivationFunctionType.Exp,
                                 bias=mb[:, j:j + 1], scale=float(SCALE))

    xt = [None] * nj

    def x_group(js):
        for j in js:
            ps = ps_px.tile([P, D], FP16, tag="px", name=f"psx{j}")
            nc.tensor.transpose(ps[0:P, 0:D], vT_sb[:, j * P:(j + 1) * P],
                                ident16[0:D, 0:D])
            x = xpool.tile([P, D + 1], FP16, tag="x", name=f"x{j}")
            nc.vector.tensor_copy(x[:, 0:D], ps[0:P, 0:D])
            nc.vector.memset(x[:, D:D + 1], 1.0)
            xt[j] = x

    def av_group(pms, num, js):
        for j in js:
            for c in range(HI // NC):
                nc.tensor.matmul(
                    num[:, c * NC:(c + 1) * NC],
                    xt[j][:],
                    pms[j][:, c * NC:(c + 1) * NC],
                    start=(j == 0), stop=(j == nj - 1),
                )

    def fin(h, num):
        osb = finp.tile([D + 1, HI], FP16, tag="osb", name=f"osb{h}")
        nc.vector.tensor_copy(osb[:], num[:])
        nc.sync.dma_start(out=out_d[:, h * HI:(h + 1) * HI], in_=osb[:])

    # ---- emission schedule ---------------------------------------------
    # front: q cols 0:256 land first, then 256:512, then k keys 0:256 --
    # the first packed score pair + exp goes on the 512-wide chunk c0 as
    # soon as those three pieces are projected.
    pairs = list(range(0, nj, 2))
    pms0 = {}
    pms1 = {}

    proj_chunk(qT_sb, wq, bq, stg["qst0a1"][:], HC, 0, 0, HC, True)
    proj_chunk(qT_sb, wq, bq, stg["qst0a2"][:], HC, 0, HC, HC, True)
    proj_chunk(kT_sb, wk, bk, stg["ksta1"][:], HC, 0, 0, HC, True)
    npre = min(max(1, KA // P // 2), len(pairs))  # pairs covered by kT 0:KA
    scores_mms(0, pairs[0], 0, 1)
    exps(0, pairs[0], pms0, 0, 1)
    if "ksta2" in stg:
        proj_chunk(kT_sb, wk, bk, stg["ksta2"][:], KA - HC, 0, HC,
                   KA - HC, True)
    for t in range(1, npre):
        scores_mms(0, pairs[t], 0, 1)
        exps(0, pairs[t], pms0, 0, 1)
    proj_chunk(qT_sb, wq, bq, stg["qst0b"][:], QB, 0, QA, QB, True)
    for t in range(npre):
        scores_mms(0, pairs[t], 1, 2)
        exps(0, pairs[t], pms0, 1, 2)
    emitted = npre
    for (o, n) in _chunks(KB, NC):
        proj_chunk(kT_sb, wk, bk, stg["kstb"][:], KB, o, KA + o, n, True)
        cov = (KA + o + n) // P
        while emitted < len(pairs) and pairs[emitted] + 1 < cov:
            scores_mms(0, pairs[emitted], 0, 2)
            exps(0, pairs[emitted], pms0, 0, 2)
            emitted += 1
        
    # remaining h0 pairs run back-to-back (PE idles briefly between them --
    # the ACT exps are the pacer); the q-half1 projection follows, timed to
    # the qst1a/qst1b arrivals.  Pump items must never sit ahead of a score
    # matmul whose data is already resident, or they head-of-line-block it.
    while emitted < len(pairs):
        scores_mms(0, pairs[emitted], 0, 2)
        exps(0, pairs[emitted], pms0, 0, 2)
        emitted += 1
    for item in proj_pumps(
            [(qT_sb, wq, bq, stg["qst1a"][:], QA, 0, HI, QA, True),
             (qT_sb, wq, bq, stg["qst1b"][:], QB, 0, HI + QA, QB, True)]):
        item()

    # ---- half 1 loop -----------------------------------------------------
    # v lands last; pump v projection + x transposes into the ACT-paced h1
    # score pairs (each slot's items sized to its piece's arrival time), and
    # run AV half0 + the half0 finalize right after the last pair's scores
    # so nothing head-of-line-blocks a score matmul.
    num0 = ps_acc.tile([D + 1, HI], F32, tag="num", name="num0")
    # ordered pump queue: per v piece -> [v proj items, x transposes,
    # AV-half0 accumulation for the unlocked key tiles]
    pq = []
    for i, (o, n) in enumerate(vsplit):
        pq.extend(proj_pumps([(vT_sb, wv, bv, stg[f"vst{i}"][:], n, 0, o, n,
                               False)]))
        xjs = list(range(o // P, (o + n) // P))
        pq.append(lambda xjs=xjs: x_group(xjs))
        pq.append(lambda xjs=xjs: av_group(pms0, num0, xjs))

    for t, j0 in enumerate(pairs):
        if t == 0:
            scores_mms(1, j0, 0, 1)
            exps(1, j0, pms1, 0, 1)
            scores_mms(1, j0, 1, 2)
            exps(1, j0, pms1, 1, 2)
        else:
            scores_mms(1, j0, 0, 2)
            exps(1, j0, pms1, 0, 2)
        if t >= 1:
            for _ in range(2):
                if pq:
                    pq.pop(0)()
    while pq:
        pq.pop(0)()
    # AV half1: num1 lives in a freed scores slot (not num0's slot), so the
    # PE can go av0 -> av1 back-to-back while the half0 finalize copy runs
    # on the DVE in parallel.  Query-chunk-outer so the first half of the
    # output ships while the second half accumulates; finalize copies ride
    # the (by then idle) Scalar engine.
    num1 = ps_mm.tile([P, HI], F32, tag="ps_mm", name="num1")[0:D + 1, :]
    fin(0, num0)

    def av1(c, js):
        for j in js:
            nc.tensor.matmul(
                num1[:, c * NC:(c + 1) * NC],
                xt[j][:],
                pms1[j][:, c * NC:(c + 1) * NC],
                start=(j == 0), stop=(j == nj - 1),
            )

    # pre-run all but the last key tile of each chunk; after the final exp
    # only 2 matmuls + the ScalarE copies + DMAs remain.
    av1(0, range(nj - 1))
    av1(1, range(nj - 1))
    for c in range(HI // NC):
        av1(c, [nj - 1])
        osb1 = finp.tile([D + 1, NC], FP16, tag="osb", name=f"osb1_{c}")
        nc.scalar.copy(osb1[:], num1[:, c * NC:(c + 1) * NC])
        nc.sync.dma_start(
            out=out_d[:, HI + c * NC:HI + (c + 1) * NC], in_=osb1[:])


_COMPILED = {}


def _get_compiled(sk2: int):
    if sk2 not in _COMPILED:
        nj = sk2 // P
        ka = min(NC, sk2)
        kb = sk2 - ka
        va = min(2 * NC, sk2)
        vb = sk2 - va
        nc = bacc.Bacc("TRN2", target_bir_lowering=False, debug=False,
                       num_devices=N_CORES)

        def din(name, shape, dt=FP16):
            return nc.dram_tensor(name, shape, dt, kind="ExternalInput").ap()

        hc = NC // 2
        ins = {
            "qst0a1": din("qst0a1", [P, NE * hc]),
            "qst0a2": din("qst0a2", [P, NE * hc]),
            "ksta1": din("ksta1", [P, NE * min(hc, ka)]),
            "qst0b": din("qst0b", [P, NE * (HI - NC)]),
            "qst1a": din("qst1a", [P, NE * NC]),
            "qst1b": din("qst1b", [P, NE * (HI - NC)]),
            "c16": din("c16", [P, 3 * NE * D]),
            "c32": din("c32", [P, nj + 3], F32),
        }
        if ka > hc:
            ins["ksta2"] = din("ksta2", [P, NE * (ka - hc)])
        if kb:
            ins["kstb"] = din("kstb", [P, NE * kb])
        for i, (o, n) in enumerate(_chunks(sk2, NC)):
            ins[f"vst{i}"] = din(f"vst{i}", [P, NE * n])
        out_d = nc.dram_tensor("out", [D + 1, S], FP16,
                               kind="ExternalOutput").ap()
        with tile.TileContext(nc) as tc:
            with ExitStack() as ctx:
                _build(tc, ins, out_d, ctx, sk2)
        nc.compile()
        _COMPILED[sk2] = nc
    return _COMPILED[sk2]


def _blob(x, lo, hi, dt):
    """[S', E] row-slice -> staging blob [P, NE*(hi-lo)] laid out as
    [partition, e-block, col]."""
    return np.ascontiguousarray(
        x[lo:hi].astype(dt).reshape(hi - lo, NE, P).transpose(2, 1, 0)
    ).reshape(P, -1)


def _wblob(w, dt):
    return (np.asarray(w, np.float32).astype(dt)
            .reshape(D, NE, P).transpose(2, 1, 0).reshape(P, NE * D))


LAST_RESULTS = None


def kernel(query, key, value, query_mask, key_mask, Wq, bq, Wk, bk, Wv, bv):
    global LAST_RESULTS
    query = np.asarray(query, dtype=np.float32)
    key = np.asarray(key, dtype=np.float32)
    value = np.asarray(value, dtype=np.float32)
    key_mask = np.asarray(key_mask)

    # compact masked keys away (they contribute exactly zero)
    keeps = [np.nonzero(key_mask[c] != 0)[0] for c in range(N_CORES)]
    nk_max = max(len(kp) for kp in keeps)
    sk2 = max(P, int(np.ceil(nk_max / P)) * P)
    sk2 = min(sk2, S)
    nj = sk2 // P
    ka = min(NC, sk2)
    va = min(2 * NC, sk2)

    w16 = np.concatenate([_wblob(Wq, np.float16), _wblob(Wk, np.float16),
                          _wblob(Wv, np.float16)], axis=1)
    c32 = np.zeros((P, nj + 3), np.float32)
    for i, b in enumerate((bq, bk, bv)):
        c32[0:D, nj + i] = np.asarray(b, np.float32).reshape(D)
        c32[D:P, nj + i] = c32[0:D, nj + i]   # row-group-64 duplicates

    in_maps = []
    for c in range(N_CORES):
        kp = keeps[c]
        nk = len(kp)
        kc = np.zeros((sk2, E), np.float32)
        vc = np.zeros((sk2, E), np.float32)
        kc[0:nk] = key[c][kp]
        vc[0:nk] = value[c][kp]
        c32c = c32.copy()
        mbias = np.full(sk2, np.float32(MASK_NEG))
        mbias[0:nk] = 0.0
        c32c[:, 0:nj] = mbias.reshape(nj, P).T
        hc = NC // 2
        im = {
            "qst0a1": _blob(query[c], 0, hc, np.float16),
            "qst0a2": _blob(query[c], hc, NC, np.float16),
            "ksta1": _blob(kc, 0, min(hc, ka), np.float16),
            "qst0b": _blob(query[c], NC, HI, np.float16),
            "qst1a": _blob(query[c], HI, HI + NC, np.float16),
            "qst1b": _blob(query[c], HI + NC, S, np.float16),
            "c16": w16,
            "c32": np.ascontiguousarray(c32c),
        }
        if ka > hc:
            im["ksta2"] = _blob(kc, hc, ka, np.float16)
        if sk2 > ka:
            im["kstb"] = _blob(kc, ka, sk2, np.float16)
        for i, (o, n) in enumerate(_chunks(sk2, NC)):
            im[f"vst{i}"] = _blob(vc, o, o + n, np.float16)
        in_maps.append(im)

    nc = _get_compiled(sk2)
    res = run_bass_kernel_spmd(nc, in_maps, core_ids=list(range(N_CORES)))
    LAST_RESULTS = res
    out = np.empty((N_CORES, S, D), np.float32)
    for c in range(N_CORES):
        o = np.asarray(res.results[c]["out"]).astype(np.float32)  # [65, S]
        out[c] = (o[0:D] / o[D:D + 1]).T
    return out
